# revision 1
# baseline (speedup 1.0000x reference)
"""GAT-style masked self-attention (B=4, N=4096, D=128) on 8 trn2 NeuronCores.

reference:
    scores = X @ X^T / sqrt(D)            [B, N, N]
    masked = where(adj > 0, scores, -1e12)
    attn   = softmax(masked, axis=2)
    out    = attn @ X                     [B, N, D]

Sharding: 8 cores <- (batch b, row-half h); each core handles 2048 rows
of one batch element against all 4096 keys. No collectives: every core
produces its own 2048x128 output slice.

Device algorithm (per core), orientation "S^T" (keys on partitions):
  - score matmul (float32r, full PE rate): psS = XT[:,k128].T @ XTrows[:,blk]
  - ACT evicts PSUM with exp fused: p = exp(scale*psS - 8)  (fp16; the -8
    keeps probs inside fp16 range and cancels in the softmax ratio)
  - DVE applies the 0/1 mask in one big 2x-mode multiply per 8-key-tile
    super group: ptm = p * adjT
  - AV matmul with the denominator fused via an appended ones-column:
      psO[rc] (+)= ptm[:, k, rc128].T @ [X_k | 1]   accumulated over k
      out = psO[:, :128] * (1 / psO[:, 128])        row-wise normalize
  - softmax shift-invariance makes a row-max pass unnecessary:
    scores*scale are bounded (~|s|<16), exp stays well inside fp32 range.
  - row blocks are software-pipelined: block i runs scores/exp/mask while
    block i-1 runs its AV matmuls (ptm double-buffered); AV matmuls are
    emitted first within each group so PE covers the ACT drain. The last
    two blocks are 256 rows so the final (unoverlapped) AV drain is short.
"""

import math
import sys

sys.path.insert(0, "/opt/trn_rl_repo")

import numpy as np

B, N, D = 4, 4096, 128
R = N // 2            # rows per core
NK = N // 128         # 32 key tiles
RB = 512              # row granularity of the host-packed mask layout
NRB = R // RB
SG = 8                # key tiles per super group (one mask DMA / mask mul)
NSG = NK // SG
SCALE = 1.0 / math.sqrt(D)
EXP_BIAS = -8.0       # exp(s*scale - 8): keeps probs in fp16 range; cancels

# row blocks (offset, size): last two halved to shorten the AV drain tail
BLOCKS = [(0, 512), (512, 512), (1024, 512), (1536, 256), (1792, 256)]

CFG = dict(
    score_dt="float16",
    p_dt="float16",
    adj_dt="float16",
    ptm_bufs=2,
    kg=2,                 # key tiles per PSUM score tile (= ACT evict batch)
    psum_s_bufs=2,
    adj_bufs=4,
)

_CACHE = {}


def _build_nc(cfg):
    from concourse import bacc
    import concourse.mybir as mybir
    from concourse.tile import TileContext

    dt = mybir.dt
    score_dt = getattr(dt, cfg["score_dt"])
    p_dt = getattr(dt, cfg["p_dt"])
    adj_dt = getattr(dt, cfg["adj_dt"])
    kg = cfg["kg"]

    nc = bacc.Bacc(None, target_bir_lowering=False)

    xt_d = nc.dram_tensor("xt", [D, N], score_dt, kind="ExternalInput")
    xtr_d = nc.dram_tensor("xtr", [D, R], score_dt, kind="ExternalInput")
    xaug_d = nc.dram_tensor("xaug", [N, D + 1], p_dt, kind="ExternalInput")
    # 0/1 mask, host-packed as [rb, key_in_tile, key_tile, row_in_block]
    adj_d = nc.dram_tensor("adjt", [NRB, 128, NK, RB], adj_dt, kind="ExternalInput")
    o_d = nc.dram_tensor("o", [R, D], dt.float32, kind="ExternalOutput")

    def adj_src(off, bs, sg0, nsg):
        rb0, r0 = off // RB, off % RB
        return adj_d[rb0, :, sg0 * SG:(sg0 + nsg) * SG, r0:r0 + bs]

    with TileContext(nc) as tc:
        with (
            tc.tile_pool(name="singles", bufs=1) as singles,
            tc.tile_pool(name="ptm", bufs=cfg["ptm_bufs"]) as ptm_pool,
            tc.tile_pool(name="adj", bufs=cfg["adj_bufs"]) as adj_pool,
            tc.tile_pool(name="pe", bufs=3) as pe_pool,
            tc.tile_pool(name="outs", bufs=4) as out_pool,
            tc.tile_pool(name="small", bufs=4) as small_pool,
            tc.tile_pool(name="psS", bufs=cfg["psum_s_bufs"], space="PSUM") as psS_pool,
            tc.tile_pool(name="psO", bufs=4, space="PSUM") as psO_pool,
        ):
            ebias = singles.tile([128, 1], mybir.dt.float32)
            nc.vector.memset(ebias[:], EXP_BIAS)
            # warm the exp table while the init DMAs stream in
            warm = small_pool.tile([128, 1], mybir.dt.float32, tag="warm")
            nc.vector.memset(warm[:], 0.0)
            warm2 = small_pool.tile([128, 1], mybir.dt.float32, tag="warm")
            nc.scalar.activation(
                warm2[:], warm[:], mybir.ActivationFunctionType.Exp, scale=1.0
            )

            # init DMAs staggered by first consumption. sync HWDGE ring:
            # xt/xtr pieces the first score matmuls need, then the rest of
            # xt. SWDGE ring (independent): first mask chunk, rest of xtr,
            # xaug.
            xt_sb = singles.tile([D, N], score_dt)
            xtr_sb = singles.tile([D, R], score_dt)
            nc.sync.dma_start(out=xtr_sb[:, 0:512], in_=xtr_d[:, 0:512])
            nc.sync.dma_start(out=xt_sb[:, 0:1024], in_=xt_d[:, 0:1024])
            nc.sync.dma_start(out=xt_sb[:, 1024:2048], in_=xt_d[:, 1024:2048])
            adj0 = adj_pool.tile([128, SG, RB], adj_dt, tag="adj", name="adj_0_0")
            nc.sync.dma_start(out=adj0[:, :, 0:BLOCKS[0][1]],
                              in_=adj_src(0, BLOCKS[0][1], 0, 1))
            nc.sync.dma_start(out=xtr_sb[:, 512:1024], in_=xtr_d[:, 512:1024])
            nc.sync.dma_start(out=xt_sb[:, 2048:4096], in_=xt_d[:, 2048:4096])
            xaug_sb = singles.tile([128, NK, D + 1], p_dt)
            nc.gpsimd.dma_start(
                out=xaug_sb[:],
                in_=xaug_d[:, :].rearrange("(t p) d -> p t d", p=128),
            )
            nc.gpsimd.dma_start(out=xtr_sb[:, 1024:2048], in_=xtr_d[:, 1024:2048])

            NB = len(BLOCKS)
            ptm_prev = None
            bs_prev = None
            off_prev = None
            for phase in range(NB + 1):
                ptm_cur = None
                psO = None
                adj_sbs = []
                if phase < NB:
                    off, bs = BLOCKS[phase]
                    ptm_cur = ptm_pool.tile([128, NK, bs], p_dt, tag="ptm",
                                            name=f"ptm_{phase}")
                    if phase == 0:
                        # 1MB chunks: the first block's mask demand starts
                        # before the DMA stream has caught up
                        adj_sbs.append((adj0, 0))
                        for sg in range(1, NSG):
                            a = adj_pool.tile([128, SG, bs], adj_dt, tag="adj",
                                              name=f"adj_{phase}_{sg}")
                            nc.sync.dma_start(out=a[:],
                                              in_=adj_src(off, bs, sg, 1))
                            adj_sbs.append((a, 0))
                    else:
                        # 2MB chunks for DMA efficiency
                        per = 1 if bs == 512 else 2   # ~1MB per chunk
                        for c in range(NSG // per):
                            a = adj_pool.tile([128, per * SG, bs], adj_dt,
                                              tag="adj", name=f"adj_{phase}_{c}")
                            nc.sync.dma_start(out=a[:],
                                              in_=adj_src(off, bs, c * per, per))
                            for i in range(per):
                                adj_sbs.append((a, i))
                if phase >= 1:
                    psO = [
                        psO_pool.tile(
                            [128, D + 1], mybir.dt.float32,
                            tag="psO", name=f"psO_{phase}_{rc}",
                        )
                        for rc in range(bs_prev // 128)
                    ]

                if phase == NB:
                    # drain: rc-major AV bursts so each psO finishes early
                    # and its normalize/store overlaps the next burst
                    for rc in range(bs_prev // 128):
                        for k in range(NK):
                            nc.tensor.matmul(
                                psO[rc][:, :],
                                lhsT=ptm_prev[:, k, rc * 128:(rc + 1) * 128],
                                rhs=xaug_sb[:, k, :],
                                start=(k == 0),
                                stop=(k == NK - 1),
                            )
                        recip = small_pool.tile([128, 1], mybir.dt.float32,
                                                tag="recip", name=f"recipd_{rc}")
                        nc.vector.reciprocal(recip[:], psO[rc][:, D:D + 1])
                        o_sb = out_pool.tile([128, D], mybir.dt.float32, tag="o",
                                             name=f"od_{rc}")
                        nc.vector.tensor_scalar_mul(o_sb[:], psO[rc][:, 0:D],
                                                    recip[:])
                        r0 = off_prev + rc * 128
                        nc.sync.dma_start(out=o_d[r0:r0 + 128, :], in_=o_sb[:])
                    break

                kg_b = kg * (RB // bs)   # keep kg_b*bs = 1024 elems per evict
                for sg in range(NSG):
                    pet = pe_pool.tile([128, SG, bs], p_dt, tag="pe",
                                       name=f"pe_{phase}_{sg}")
                    for kgi in range(SG // kg_b):
                        # AV matmuls for the previous block first: PE has
                        # work while ACT drains this group's scores.
                        if phase >= 1:
                            for j in range(kg_b):
                                k = sg * SG + kgi * kg_b + j
                                for rc in range(bs_prev // 128):
                                    nc.tensor.matmul(
                                        psO[rc][:, :],
                                        lhsT=ptm_prev[:, k, rc * 128:(rc + 1) * 128],
                                        rhs=xaug_sb[:, k, :],
                                        start=(k == 0),
                                        stop=(k == NK - 1),
                                    )
                        ps = psS_pool.tile([128, kg_b, bs], mybir.dt.float32,
                                           tag="psS", name=f"psS_{phase}_{sg}_{kgi}")
                        for j in range(kg_b):
                            k = sg * SG + kgi * kg_b + j
                            nc.tensor.matmul(
                                ps[:, j, :],
                                lhsT=xt_sb[:, k * 128:(k + 1) * 128],
                                rhs=xtr_sb[:, off:off + bs],
                                start=True,
                                stop=True,
                            )
                        # evict PSUM with exp fused; mask comes after
                        nc.scalar.activation(
                            pet[:, kgi * kg_b:(kgi + 1) * kg_b, :],
                            ps[:, :, :],
                            mybir.ActivationFunctionType.Exp,
                            bias=ebias[:],
                            scale=SCALE,
                        )
                    k0 = sg * SG
                    a, si = adj_sbs[sg]
                    nc.vector.tensor_mul(
                        ptm_cur[:, k0:k0 + SG, :],
                        pet[:, :, :],
                        a[:, si * SG:(si + 1) * SG, 0:bs],
                    )
                if phase >= 1:
                    for rc in range(bs_prev // 128):
                        recip = small_pool.tile([128, 1], mybir.dt.float32,
                                                tag="recip",
                                                name=f"recip_{phase}_{rc}")
                        nc.vector.reciprocal(recip[:], psO[rc][:, D:D + 1])
                        o_sb = out_pool.tile([128, D], mybir.dt.float32, tag="o",
                                             name=f"o_{phase}_{rc}")
                        nc.vector.tensor_scalar_mul(o_sb[:], psO[rc][:, 0:D],
                                                    recip[:])
                        r0 = off_prev + rc * 128
                        nc.sync.dma_start(out=o_d[r0:r0 + 128, :], in_=o_sb[:])
                ptm_prev = ptm_cur
                bs_prev = bs
                off_prev = off
    nc.finalize()
    return nc


def _get_nc():
    key = tuple(sorted(CFG.items()))
    if key not in _CACHE:
        _CACHE[key] = _build_nc(CFG)
    return _CACHE[key]


def _np_dt(name):
    import ml_dtypes

    return {
        "float32": np.float32,
        "float32r": np.float32,
        "bfloat16": ml_dtypes.bfloat16,
        "float16": np.float16,
    }[name]


def make_in_maps(input, adj):
    """Host-side shard/layout prep: one input map per core."""
    input = np.asarray(input, dtype=np.float32)
    adj = np.asarray(adj)
    score_np = _np_dt(CFG["score_dt"])
    p_np = _np_dt(CFG["p_dt"])
    adj_np = _np_dt(CFG["adj_dt"])

    in_maps = []
    for core in range(8):
        b, h = core // 2, core % 2
        xb = input[b]                                    # [N, D]
        xt = np.ascontiguousarray(xb.T).astype(score_np, copy=False)
        xtr = np.ascontiguousarray(xb.T[:, h * R:(h + 1) * R]).astype(
            score_np, copy=False
        )
        xaug = np.concatenate([xb, np.ones((N, 1), np.float32)], axis=1)
        xaug = np.ascontiguousarray(xaug).astype(p_np)
        s = adj[b][h * R:(h + 1) * R, :]                 # [R rows, N cols]
        # multiplicative 0/1 mask; adjt[rb, p, k, r] = (s[rb*512+r, k*128+p]>0)
        adjt = np.ascontiguousarray(
            (s > 0).astype(adj_np).reshape(NRB, RB, NK, 128).transpose(0, 3, 2, 1)
        )
        in_maps.append({"xt": xt, "xtr": xtr, "xaug": xaug, "adjt": adjt})
    return in_maps


def run_device(in_maps, trace=False, trace_cores=None):
    import concourse.bass_utils as bass_utils

    if trace:
        bass_utils.upload_artifacts = lambda tmpdir: ""  # no bucket in sandbox
    nc = _get_nc()
    return bass_utils.run_bass_kernel_spmd(
        nc, in_maps, list(range(8)), trace=trace, trace_cores=trace_cores
    )


def kernel(input, adj):
    res = run_device(make_in_maps(input, adj))
    out = np.empty((B, N, D), dtype=np.float32)
    for core in range(8):
        b, h = core // 2, core % 2
        out[b, h * R:(h + 1) * R, :] = res.results[core]["o"]
    return out



# revision 6
# speedup vs baseline: 1.2529x; 1.2529x over previous
"""GAT-style masked self-attention (B=4, N=4096, D=128) on 8 trn2 NeuronCores.

reference:
    scores = X @ X^T / sqrt(D)            [B, N, N]
    masked = where(adj > 0, scores, -1e12)
    attn   = softmax(masked, axis=2)
    out    = attn @ X                     [B, N, D]

Sharding: 8 cores <- (batch b, row-half h); each core handles 2048 rows
of one batch element against all 4096 keys. No collectives.

Device algorithm (per core), orientation "S^T" (keys on partitions):
  - score matmul (fp16, full PE rate): psS = XT[:,k128].T @ XTrows[:,blk].
    X is host-prescaled by sqrt(ALPHA) so psS = raw_dot*ALPHA, which is
    directly the fast-exp fixed-point argument.
  - eviction with exp, split across engines by key group:
      sg 0..2: ACT evicts with exp fused: p = exp(psS*(SCALE/ALPHA) - 8)
      sg 3:    DVE fast-exp (Schraudolph): u16 = round(psS + C); the u16
               bit pattern read as fp16 approximates exp(z)  (z<-10.4
               rounds/clamps to ~0, which is negligible mass)
  - mask is HOST BIT-PACKED 8 rows per uint16 word: bit i of word j in a
    512-row chunk is row i*64+j. DVE unpacks with one tensor_scalar
    bitwise_and per bit (4x DVE mode): m = words & (1<<i) in {0, 2^i}.
    The 2^i factor is constant per ROW, so it cancels in the softmax
    ratio (numerator and ones-column denominator share it).
  - masked probs: ptm = p * m via mixed-dtype tensor_tensor mult
    (fp16 x uint16 -> fp16, 2x mode), split between DVE and GPSIMD.
  - AV matmul with the denominator fused via an appended ones-column:
      psO[rc] (+)= ptm[:, k, rc128].T @ [X_k | 1]   accumulated over k
      out = psO[:, :128] * (1 / psO[:, 128])  (recip on DVE, the
      normalize multiply on ACT as a Copy activation with scale AP)
  - row blocks are software-pipelined as in the baseline: block i runs
    scores/exp/mask while block i-1 runs its AV matmuls; AV matmuls are
    emitted first within each group so PE covers the eviction drain.
"""

import math
import sys

sys.path.insert(0, "/opt/trn_rl_repo")

import numpy as np

B, N, D = 4, 4096, 128
R = N // 2            # rows per core
NK = N // 128         # 32 key tiles
RB = 512              # row granularity of the host-packed mask layout
NRB = R // RB
SG = 8                # key tiles per super group
NSG = NK // SG
SCALE = 1.0 / math.sqrt(D)
EXP_BIAS = -8.0

# fast-exp constants: exp(z) ~ f16_bits(round(z*1024/ln2 + 15360 - ADJ))
LOG2_SC = (1 << 10) / math.log(2.0)     # 1477.3196
ALPHA = SCALE * LOG2_SC                 # psS = raw_dot * ALPHA
SQ_ALPHA = math.sqrt(ALPHA)             # host pre-scale per score operand
FEXP_ADJ = 50.0
FEXP_C = float((15 << 10) + EXP_BIAS * LOG2_SC - FEXP_ADJ)
ACT_SCALE = SCALE / ALPHA               # makes ACT see raw_dot*SCALE

# row blocks (offset, size): last two halved to shorten the AV drain tail
BLOCKS = [(0, 512), (512, 512), (1024, 512), (1536, 256), (1792, 256)]

# engine split knobs
FEXP_SGS = (3,)            # key super-groups evicted by DVE fast-exp
# mask-mul halves assigned to GPSIMD per 512-block: (sg, half) pairs
POOL_MULS_512 = {(0, 0), (0, 1), (2, 0)}
POOL_MULS_256 = {(0, 0)}   # per 256-block: (sg, 0) full-block muls

CFG = dict(kg=2, psum_s_bufs=2)

_CACHE = {}


def _build_nc(cfg):
    from concourse import bacc
    import concourse.mybir as mybir
    from concourse.tile import TileContext

    dt = mybir.dt
    kg = cfg["kg"]

    nc = bacc.Bacc(None, target_bir_lowering=False)

    xt_d = nc.dram_tensor("xt", [D, N], dt.float16, kind="ExternalInput")
    xtr_d = nc.dram_tensor("xtr", [D, R], dt.float16, kind="ExternalInput")
    xaug_d = nc.dram_tensor("xaug", [N, D + 1], dt.bfloat16, kind="ExternalInput")
    # bit-packed 0/1 mask: words[rb, key_in_tile, key_tile, j]; bit i of
    # word j is row rb*512 + i*64 + j
    w_d = nc.dram_tensor("words", [NRB, 128, NK, 64], dt.uint16,
                         kind="ExternalInput")
    o_d = nc.dram_tensor("o", [R, D], dt.float32, kind="ExternalOutput")

    with TileContext(nc) as tc:
        with (
            tc.tile_pool(name="singles", bufs=1) as singles,
            tc.tile_pool(name="words", bufs=2) as w_pool,
            tc.tile_pool(name="mu", bufs=2) as mu_pool,
            tc.tile_pool(name="ptm", bufs=2) as ptm_pool,
            tc.tile_pool(name="pe", bufs=3) as pe_pool,
            tc.tile_pool(name="pef", bufs=3) as pef_pool,
            tc.tile_pool(name="outs", bufs=4) as out_pool,
            tc.tile_pool(name="small", bufs=4) as small_pool,
            tc.tile_pool(name="psS", bufs=cfg["psum_s_bufs"], space="PSUM") as psS_pool,
            tc.tile_pool(name="psO", bufs=4, space="PSUM") as psO_pool,
        ):
            ebias = singles.tile([128, 1], mybir.dt.float32)
            nc.vector.memset(ebias[:], EXP_BIAS)
            # warm the exp table while the init DMAs stream in
            warm = small_pool.tile([128, 1], mybir.dt.float32, tag="warm")
            nc.vector.memset(warm[:], 0.0)
            warm2 = small_pool.tile([128, 1], mybir.dt.float32, tag="warm")
            nc.scalar.activation(
                warm2[:], warm[:], mybir.ActivationFunctionType.Exp, scale=1.0
            )

            # init DMAs staggered by first consumption. sync HWDGE ring:
            # xt/xtr pieces the first score matmuls need, then the rest.
            # SWDGE ring (gpsimd, independent): first words chunk, rest of
            # xtr, xaug.
            xt_sb = singles.tile([D, N], dt.float16)
            xtr_sb = singles.tile([D, R], dt.float16)
            nc.sync.dma_start(out=xtr_sb[:, 0:512], in_=xtr_d[:, 0:512])
            nc.sync.dma_start(out=xt_sb[:, 0:1024], in_=xt_d[:, 0:1024])
            nc.sync.dma_start(out=xt_sb[:, 1024:2048], in_=xt_d[:, 1024:2048])
            w_tiles = {}
            w_tiles[0] = w_pool.tile([128, NK, 64], dt.uint16, tag="w",
                                     name="w_0")
            nc.gpsimd.dma_start(out=w_tiles[0][:], in_=w_d[0])
            nc.sync.dma_start(out=xtr_sb[:, 512:1024], in_=xtr_d[:, 512:1024])
            nc.sync.dma_start(out=xt_sb[:, 2048:4096], in_=xt_d[:, 2048:4096])
            xaug_sb = singles.tile([128, NK, D + 1], dt.bfloat16)
            nc.gpsimd.dma_start(
                out=xaug_sb[:],
                in_=xaug_d[:, :].rearrange("(t p) d -> p t d", p=128),
            )
            nc.gpsimd.dma_start(out=xtr_sb[:, 1024:2048], in_=xtr_d[:, 1024:2048])

            NB = len(BLOCKS)
            ptm_prev = None
            pet_prev = {}
            bs_prev = None
            off_prev = None
            for phase in range(NB + 1):
                ptm_cur = None
                psO = None
                if phase < NB:
                    off, bs = BLOCKS[phase]
                    rb = off // RB
                    bit0 = (off % RB) // 64      # first bit index in chunk
                    nbits = bs // 64
                    # prefetch words for the next 512-chunk
                    rb_next = (off + bs) // RB
                    if phase + 1 < NB and rb_next != rb and rb_next not in w_tiles:
                        w_tiles[rb_next] = w_pool.tile(
                            [128, NK, 64], dt.uint16, tag="w", name=f"w_{rb_next}"
                        )
                        nc.gpsimd.dma_start(out=w_tiles[rb_next][:],
                                            in_=w_d[rb_next])
                    # unpack this block's mask bits: m in {0, 2^i} (u16)
                    m_u = mu_pool.tile([128, NK, bs], dt.uint16, tag="mu",
                                       name=f"mu_{phase}")
                    wt = w_tiles[rb]
                    for i in range(nbits):
                        nc.vector.tensor_scalar(
                            m_u[:, :, i * 64:(i + 1) * 64],
                            wt[:],
                            1 << (bit0 + i),
                            None,
                            mybir.AluOpType.bitwise_and,
                        )
                    ptm_cur = ptm_pool.tile([128, NK, bs], dt.bfloat16, tag="ptm",
                                            name=f"ptm_{phase}")
                if phase >= 1:
                    psO = [
                        psO_pool.tile(
                            [128, D + 1], mybir.dt.float32,
                            tag="psO", name=f"psO_{phase}_{rc}",
                        )
                        for rc in range(bs_prev // 128)
                    ]

                if phase == NB:
                    # drain: rc-major AV bursts so each psO finishes early
                    # and its normalize/store overlaps the next burst
                    for rc in range(bs_prev // 128):
                        for k in range(NK):
                            nc.tensor.matmul(
                                psO[rc][:, :],
                                lhsT=ptm_prev[:, k, rc * 128:(rc + 1) * 128],
                                rhs=xaug_sb[:, k, :],
                                start=(k == 0),
                                stop=(k == NK - 1),
                            )
                        recip = small_pool.tile([128, 1], mybir.dt.float32,
                                                tag="recip", name=f"recipd_{rc}")
                        nc.vector.reciprocal(recip[:], psO[rc][:, D:D + 1])
                        o_sb = out_pool.tile([128, D], mybir.dt.float32, tag="o",
                                             name=f"od_{rc}")
                        nc.scalar.activation(
                            o_sb[:], psO[rc][:, 0:D],
                            mybir.ActivationFunctionType.Copy, scale=recip[:],
                        )
                        r0 = off_prev + rc * 128
                        nc.sync.dma_start(out=o_d[r0:r0 + 128, :], in_=o_sb[:])
                    break

                kg_b = kg * (RB // bs)   # keep kg_b*bs = 1024 elems per evict
                pet_cur = {}
                pool_muls = POOL_MULS_512 if bs == 512 else POOL_MULS_256
                halves = 2 if bs == 512 else 1
                hs = bs // halves
                for sg in range(NSG):
                    use_fexp = sg in FEXP_SGS
                    if use_fexp:
                        pet = pef_pool.tile([128, SG, bs], dt.uint16, tag="pef",
                                            name=f"pef_{phase}_{sg}")
                    else:
                        pet = pe_pool.tile([128, SG, bs], dt.bfloat16, tag="pe",
                                           name=f"pe_{phase}_{sg}")
                    pet_cur[sg] = pet
                    for kgi in range(SG // kg_b):
                        # AV matmuls for the previous block first: PE has
                        # work while the eviction drains this group's scores
                        if phase >= 1:
                            for j in range(kg_b):
                                k = sg * SG + kgi * kg_b + j
                                for rc in range(bs_prev // 128):
                                    nc.tensor.matmul(
                                        psO[rc][:, :],
                                        lhsT=ptm_prev[:, k, rc * 128:(rc + 1) * 128],
                                        rhs=xaug_sb[:, k, :],
                                        start=(k == 0),
                                        stop=(k == NK - 1),
                                    )
                        ps = psS_pool.tile([128, kg_b, bs], mybir.dt.float32,
                                           tag="psS", name=f"psS_{phase}_{sg}_{kgi}")
                        for j in range(kg_b):
                            k = sg * SG + kgi * kg_b + j
                            nc.tensor.matmul(
                                ps[:, j, :],
                                lhsT=xt_sb[:, k * 128:(k + 1) * 128],
                                rhs=xtr_sb[:, off:off + bs],
                                start=True,
                                stop=True,
                            )
                        sl = pet[:, kgi * kg_b:(kgi + 1) * kg_b, :]
                        if use_fexp:
                            # fast-exp on DVE: u16 = round(psS + C), clamped
                            # below the fp16 inf/nan bit region
                            nc.vector.tensor_scalar(
                                sl, ps[:, :, :], FEXP_C, 31600.0,
                                mybir.AluOpType.add,
                                mybir.AluOpType.min,
                            )
                        else:
                            nc.scalar.activation(
                                sl, ps[:, :, :],
                                mybir.ActivationFunctionType.Exp,
                                bias=ebias[:],
                                scale=ACT_SCALE,
                            )
                    # masked probs for this super group, split DVE/GPSIMD
                    k0 = sg * SG
                    pin = pet[:].bitcast(dt.float16) if use_fexp else pet[:]
                    for h in range(halves):
                        eng = nc.gpsimd if (sg, h) in pool_muls else nc.vector
                        eng.tensor_tensor(
                            ptm_cur[:, k0:k0 + SG, h * hs:(h + 1) * hs],
                            pin[:, :, h * hs:(h + 1) * hs],
                            m_u[:, k0:k0 + SG, h * hs:(h + 1) * hs],
                            mybir.AluOpType.mult,
                        )
                if phase >= 1:
                    for rc in range(bs_prev // 128):
                        recip = small_pool.tile([128, 1], mybir.dt.float32,
                                                tag="recip",
                                                name=f"recip_{phase}_{rc}")
                        nc.vector.reciprocal(recip[:], psO[rc][:, D:D + 1])
                        o_sb = out_pool.tile([128, D], mybir.dt.float32, tag="o",
                                             name=f"o_{phase}_{rc}")
                        nc.scalar.activation(
                            o_sb[:], psO[rc][:, 0:D],
                            mybir.ActivationFunctionType.Copy, scale=recip[:],
                        )
                        r0 = off_prev + rc * 128
                        nc.sync.dma_start(out=o_d[r0:r0 + 128, :], in_=o_sb[:])
                ptm_prev = ptm_cur
                pet_prev = pet_cur
                bs_prev = bs
                off_prev = off
    nc.finalize()
    return nc


def _get_nc():
    key = tuple(sorted(CFG.items()))
    if key not in _CACHE:
        _CACHE[key] = _build_nc(CFG)
    return _CACHE[key]


def make_in_maps(input, adj):
    """Host-side shard/layout prep: one input map per core."""
    input = np.asarray(input, dtype=np.float32)
    adj = np.asarray(adj)

    in_maps = []
    for core in range(8):
        b, h = core // 2, core % 2
        xb = input[b]                                    # [N, D]
        xs = (xb.T * SQ_ALPHA).astype(np.float16)        # pre-scaled scores
        xt = np.ascontiguousarray(xs)
        xtr = np.ascontiguousarray(xs[:, h * R:(h + 1) * R])
        import ml_dtypes
        xaug = np.concatenate([xb, np.ones((N, 1), np.float32)], axis=1)
        xaug = np.ascontiguousarray(xaug).astype(ml_dtypes.bfloat16)
        s = adj[b][h * R:(h + 1) * R, :] > 0             # [R rows, N cols]
        # words[rb, p, k, j]: bit i = mask row rb*512+i*64+j, key k*128+p
        sb = s.reshape(NRB, 8, 64, NK, 128)              # [rb, i, j, k, p]
        words = (
            sb.astype(np.uint16) << np.arange(8, dtype=np.uint16)[None, :, None, None, None]
        ).sum(axis=1, dtype=np.uint16)                   # [rb, j, k, p]
        words = np.ascontiguousarray(words.transpose(0, 3, 2, 1))  # [rb,p,k,j]
        in_maps.append({"xt": xt, "xtr": xtr, "xaug": xaug, "words": words})
    return in_maps


def run_device(in_maps, trace=False, trace_cores=None):
    import concourse.bass_utils as bass_utils

    if trace:
        bass_utils.upload_artifacts = lambda tmpdir: ""  # no bucket in sandbox
    nc = _get_nc()
    return bass_utils.run_bass_kernel_spmd(
        nc, in_maps, list(range(8)), trace=trace, trace_cores=trace_cores
    )


def kernel(input, adj):
    res = run_device(make_in_maps(input, adj))
    out = np.empty((B, N, D), dtype=np.float32)
    for core in range(8):
        b, h = core // 2, core % 2
        out[b, h * R:(h + 1) * R, :] = res.results[core]["o"]
    return out


# revision 7
# speedup vs baseline: 1.3131x; 1.0480x over previous
"""GAT-style masked self-attention (B=4, N=4096, D=128) on 8 trn2 NeuronCores.

reference:
    scores = X @ X^T / sqrt(D)            [B, N, N]
    masked = where(adj > 0, scores, -1e12)
    attn   = softmax(masked, axis=2)
    out    = attn @ X                     [B, N, D]

Sharding: 8 cores <- (batch b, row-half h); each core handles 2048 rows
of one batch element against all 4096 keys. No collectives.

Device algorithm (per core), orientation "S^T" (keys on partitions).
X is host-prescaled by sqrt(ALPHA) so the score matmul's PSUM value is
directly the fast-exp fixed-point argument psS = raw_dot*ALPHA.

Per row-block, key super-groups are processed in order (3, 0, 1, 2):

  sg 3 (keys 3072:4095), DVE fast-exp with the mask baked in:
      u16 = convert(psS + madd)   [one tensor_tensor add]
    madd (host int16) = unmasked ? (C + 1024*bit(row)) : -32768.
    Unmasked: Schraudolph fast exp -- the u16 bit pattern read as fp16
    is 2^bit(row) * exp(raw_dot*SCALE - 8); masked: the sum is negative
    and the u16 convert clamps to 0. The host clamps diagonal entries
    so the sum stays below the fp16 inf bit region. The result feeds
    the AV matmul directly (bitcast fp16), no separate mask multiply.

  sg 0..2 (keys 0:3071), ACT exp + bit-packed mask:
    - ACT evicts PSUM with exp fused: p = exp(psS*(SCALE/ALPHA) - 8),
      output bfloat16 (wide exponent absorbs the mask row constants)
    - mask host-packed 8 rows per uint16 word (bit i of word j in a
      512-row chunk = row i*64+j); DVE unpacks with one tensor_scalar
      bitwise_and per bit (4x mode): m = words & (1<<i) in {0, 2^i}
    - ptm = p * m via mixed tensor_tensor mult (bf16 x u16 -> bf16, 2x),
      split between DVE and GPSIMD

  The per-row constant 2^bit(row) is identical across all keys of a row
  (additively in sg3's exponent, multiplicatively in sg0-2), so it
  cancels in the softmax ratio.

  AV matmul with the denominator fused via an appended ones-column:
      psO[rc] (+)= w_k.T @ [X_k | 1] over k in (24..31, 0..23)
      w_k = pef (fp16 bitcast) for sg3, ptm (bf16) for sg0..2
      out = psO[:, :128] * (1 / psO[:, 128]) -- recip on DVE, the
      normalize multiply on ACT (Copy activation with scale AP)

  Row blocks are software-pipelined: block i runs scores/exp/mask while
  block i-1 runs its AV matmuls; AV matmuls are emitted first within
  each group so PE covers the eviction drain.
"""

import math
import sys

sys.path.insert(0, "/opt/trn_rl_repo")

import numpy as np

B, N, D = 4, 4096, 128
R = N // 2            # rows per core
NK = N // 128         # 32 key tiles
NK_M = 24             # key tiles masked via bit-unpack (sg 0..2)
RB = 512              # row granularity of the host-packed mask layout
NRB = R // RB
SG = 8                # key tiles per super group
NSG = NK // SG
SG_ORDER = (3, 0, 1, 2)
SCALE = 1.0 / math.sqrt(D)
EXP_BIAS = -8.0

# fast-exp: exp(z) ~ f16_bits(round(z*1024/ln2 + 15360 - ADJ))
LOG2_SC = (1 << 10) / math.log(2.0)     # 1477.3196
ALPHA = SCALE * LOG2_SC                 # psS = raw_dot * ALPHA
SQ_ALPHA = math.sqrt(ALPHA)             # host pre-scale per score operand
FEXP_ADJ = 50.0
FEXP_C = float(round((15 << 10) + EXP_BIAS * LOG2_SC - FEXP_ADJ))
ACT_SCALE = SCALE / ALPHA               # makes ACT see raw_dot*SCALE

# row blocks (offset, size): last two halved to shorten the AV drain tail
BLOCKS = [(0, 512), (512, 512), (1024, 512), (1536, 256), (1792, 256)]

# mask-mul units assigned to GPSIMD: (sg, half) per 512-block, (sg, 0)
# per 256-block; the rest go to DVE
POOL_MULS_512 = {(0, 0), (1, 1), (2, 0)}
POOL_MULS_256 = {(1, 0)}

CFG = dict(kg=2, psum_s_bufs=2)

_CACHE = {}


def _build_nc(cfg):
    from concourse import bacc
    import concourse.mybir as mybir
    from concourse.tile import TileContext

    dt = mybir.dt
    kg = cfg["kg"]

    nc = bacc.Bacc(None, target_bir_lowering=False)

    xt_d = nc.dram_tensor("xt", [D, N], dt.float16, kind="ExternalInput")
    xtr_d = nc.dram_tensor("xtr", [D, R], dt.float16, kind="ExternalInput")
    xaug_d = nc.dram_tensor("xaug", [N, D + 1], dt.bfloat16, kind="ExternalInput")
    # bit-packed 0/1 mask for key tiles 0..23
    w_d = nc.dram_tensor("words", [NRB, 128, NK_M, 64], dt.uint16,
                         kind="ExternalInput")
    # additive fast-exp mask for key tiles 24..31
    madd_d = nc.dram_tensor("madd", [128, SG, R], dt.int16,
                            kind="ExternalInput")
    o_d = nc.dram_tensor("o", [R, D], dt.float32, kind="ExternalOutput")

    with TileContext(nc) as tc:
        with (
            tc.tile_pool(name="singles", bufs=1) as singles,
            tc.tile_pool(name="words", bufs=2) as w_pool,
            tc.tile_pool(name="madd", bufs=2) as madd_pool,
            tc.tile_pool(name="mu", bufs=2) as mu_pool,
            tc.tile_pool(name="ptm", bufs=2) as ptm_pool,
            tc.tile_pool(name="pe", bufs=3) as pe_pool,
            tc.tile_pool(name="pef", bufs=2) as pef_pool,
            tc.tile_pool(name="outs", bufs=4) as out_pool,
            tc.tile_pool(name="small", bufs=4) as small_pool,
            tc.tile_pool(name="psS", bufs=cfg["psum_s_bufs"], space="PSUM") as psS_pool,
            tc.tile_pool(name="psO", bufs=4, space="PSUM") as psO_pool,
        ):
            ebias = singles.tile([128, 1], mybir.dt.float32)
            nc.vector.memset(ebias[:], EXP_BIAS)
            # warm the exp table while the init DMAs stream in
            warm = small_pool.tile([128, 1], mybir.dt.float32, tag="warm")
            nc.vector.memset(warm[:], 0.0)
            warm2 = small_pool.tile([128, 1], mybir.dt.float32, tag="warm")
            nc.scalar.activation(
                warm2[:], warm[:], mybir.ActivationFunctionType.Exp, scale=1.0
            )

            # init DMAs staggered by first consumption. sg3 (keys 3072:)
            # is processed first, so its xt slice and madd chunk lead.
            xt_sb = singles.tile([D, N], dt.float16)
            xtr_sb = singles.tile([D, R], dt.float16)
            nc.sync.dma_start(out=xtr_sb[:, 0:512], in_=xtr_d[:, 0:512])
            nc.sync.dma_start(out=xt_sb[:, 3072:4096], in_=xt_d[:, 3072:4096])
            madd_tiles = {}
            madd_tiles[0] = madd_pool.tile([128, SG, 512], dt.int16,
                                           tag="madd", name="madd_0")
            nc.gpsimd.dma_start(out=madd_tiles[0][:], in_=madd_d[:, :, 0:512])
            nc.sync.dma_start(out=xt_sb[:, 0:1536], in_=xt_d[:, 0:1536])
            w_tiles = {}
            w_tiles[0] = w_pool.tile([128, NK_M, 64], dt.uint16, tag="w",
                                     name="w_0")
            nc.gpsimd.dma_start(out=w_tiles[0][:], in_=w_d[0])
            nc.sync.dma_start(out=xtr_sb[:, 512:1024], in_=xtr_d[:, 512:1024])
            nc.sync.dma_start(out=xt_sb[:, 1536:3072], in_=xt_d[:, 1536:3072])
            xaug_sb = singles.tile([128, NK, D + 1], dt.bfloat16)
            nc.gpsimd.dma_start(
                out=xaug_sb[:],
                in_=xaug_d[:, :].rearrange("(t p) d -> p t d", p=128),
            )
            nc.gpsimd.dma_start(out=xtr_sb[:, 1024:2048], in_=xtr_d[:, 1024:2048])

            NB = len(BLOCKS)
            ptm_prev = None
            pef_prev = None
            bs_prev = None
            off_prev = None

            def av_k_seq():
                for sgi in SG_ORDER:
                    for j in range(SG):
                        yield sgi * SG + j

            K_FIRST = SG_ORDER[0] * SG
            K_LAST = SG_ORDER[-1] * SG + SG - 1

            def emit_av(psO, k, rc):
                if k >= NK_M:
                    lhsT = pef_prev[:, k - NK_M, rc * 128:(rc + 1) * 128].bitcast(
                        dt.float16
                    )
                else:
                    lhsT = ptm_prev[:, k, rc * 128:(rc + 1) * 128]
                nc.tensor.matmul(
                    psO[rc][:, :],
                    lhsT=lhsT,
                    rhs=xaug_sb[:, k, :],
                    start=(k == K_FIRST),
                    stop=(k == K_LAST),
                )

            for phase in range(NB + 1):
                ptm_cur = None
                pef_cur = None
                psO = None
                if phase < NB:
                    off, bs = BLOCKS[phase]
                    rb = off // RB
                    bit0 = (off % RB) // 64
                    nbits = bs // 64
                    # prefetch next block's madd / words
                    if phase + 1 < NB:
                        off_n, bs_n = BLOCKS[phase + 1]
                        madd_tiles[phase + 1] = madd_pool.tile(
                            [128, SG, bs_n], dt.int16, tag="madd",
                            name=f"madd_{phase + 1}"
                        )
                        nc.gpsimd.dma_start(
                            out=madd_tiles[phase + 1][:],
                            in_=madd_d[:, :, off_n:off_n + bs_n],
                        )
                        rb_n = off_n // RB
                        if rb_n != rb and rb_n not in w_tiles:
                            w_tiles[rb_n] = w_pool.tile(
                                [128, NK_M, 64], dt.uint16, tag="w",
                                name=f"w_{rb_n}"
                            )
                            nc.gpsimd.dma_start(out=w_tiles[rb_n][:],
                                                in_=w_d[rb_n])
                    # unpack this block's mask bits (key tiles 0..23)
                    m_u = mu_pool.tile([128, NK_M, bs], dt.uint16, tag="mu",
                                       name=f"mu_{phase}")
                    wt = w_tiles[rb]
                    for i in range(nbits):
                        nc.vector.tensor_scalar(
                            m_u[:, :, i * 64:(i + 1) * 64],
                            wt[:],
                            1 << (bit0 + i),
                            None,
                            mybir.AluOpType.bitwise_and,
                        )
                    ptm_cur = ptm_pool.tile([128, NK_M, bs], dt.bfloat16,
                                            tag="ptm", name=f"ptm_{phase}")
                    pef_cur = pef_pool.tile([128, SG, bs], dt.uint16,
                                            tag="pef", name=f"pef_{phase}")
                    madd_t = madd_tiles.get(phase)
                if phase >= 1:
                    psO = [
                        psO_pool.tile(
                            [128, D + 1], mybir.dt.float32,
                            tag="psO", name=f"psO_{phase}_{rc}",
                        )
                        for rc in range(bs_prev // 128)
                    ]

                if phase == NB:
                    # drain: rc-major AV bursts so each psO finishes early
                    for rc in range(bs_prev // 128):
                        for k in av_k_seq():
                            emit_av(psO, k, rc)
                        recip = small_pool.tile([128, 1], mybir.dt.float32,
                                                tag="recip", name=f"recipd_{rc}")
                        nc.vector.reciprocal(recip[:], psO[rc][:, D:D + 1])
                        o_sb = out_pool.tile([128, D], mybir.dt.float32, tag="o",
                                             name=f"od_{rc}")
                        nc.scalar.activation(
                            o_sb[:], psO[rc][:, 0:D],
                            mybir.ActivationFunctionType.Copy, scale=recip[:],
                        )
                        r0 = off_prev + rc * 128
                        nc.sync.dma_start(out=o_d[r0:r0 + 128, :], in_=o_sb[:])
                    break

                kg_b = kg * (RB // bs)   # keep kg_b*bs = 1024 elems per evict
                pool_muls = POOL_MULS_512 if bs == 512 else POOL_MULS_256
                halves = 2 if bs == 512 else 1
                hs = bs // halves
                av_iter = iter(av_k_seq())
                for sg in SG_ORDER:
                    use_fexp = sg == 3
                    if use_fexp:
                        pet = pef_cur
                    else:
                        pet = pe_pool.tile([128, SG, bs], dt.bfloat16, tag="pe",
                                           name=f"pe_{phase}_{sg}")
                    for kgi in range(SG // kg_b):
                        # AV matmuls for the previous block first: PE has
                        # work while the eviction drains this group's scores
                        if phase >= 1:
                            for _ in range(kg_b):
                                k = next(av_iter)
                                for rc in range(bs_prev // 128):
                                    emit_av(psO, k, rc)
                        ps = psS_pool.tile([128, kg_b, bs], mybir.dt.float32,
                                           tag="psS", name=f"psS_{phase}_{sg}_{kgi}")
                        for j in range(kg_b):
                            k = sg * SG + kgi * kg_b + j
                            nc.tensor.matmul(
                                ps[:, j, :],
                                lhsT=xt_sb[:, k * 128:(k + 1) * 128],
                                rhs=xtr_sb[:, off:off + bs],
                                start=True,
                                stop=True,
                            )
                        sl = pet[:, kgi * kg_b:(kgi + 1) * kg_b, :]
                        if use_fexp:
                            # fast-exp with mask & row constant baked into
                            # madd; u16 convert clamps negatives to 0
                            nc.vector.tensor_tensor(
                                sl, ps[:, :, :],
                                madd_t[:, kgi * kg_b:(kgi + 1) * kg_b, :],
                                mybir.AluOpType.add,
                            )
                        else:
                            nc.scalar.activation(
                                sl, ps[:, :, :],
                                mybir.ActivationFunctionType.Exp,
                                bias=ebias[:],
                                scale=ACT_SCALE,
                            )
                    if not use_fexp:
                        k0 = sg * SG
                        for h in range(halves):
                            eng = nc.gpsimd if (sg, h) in pool_muls else nc.vector
                            eng.tensor_tensor(
                                ptm_cur[:, k0:k0 + SG, h * hs:(h + 1) * hs],
                                pet[:, :, h * hs:(h + 1) * hs],
                                m_u[:, k0:k0 + SG, h * hs:(h + 1) * hs],
                                mybir.AluOpType.mult,
                            )
                if phase >= 1:
                    for rc in range(bs_prev // 128):
                        recip = small_pool.tile([128, 1], mybir.dt.float32,
                                                tag="recip",
                                                name=f"recip_{phase}_{rc}")
                        nc.vector.reciprocal(recip[:], psO[rc][:, D:D + 1])
                        o_sb = out_pool.tile([128, D], mybir.dt.float32, tag="o",
                                             name=f"o_{phase}_{rc}")
                        nc.scalar.activation(
                            o_sb[:], psO[rc][:, 0:D],
                            mybir.ActivationFunctionType.Copy, scale=recip[:],
                        )
                        r0 = off_prev + rc * 128
                        nc.sync.dma_start(out=o_d[r0:r0 + 128, :], in_=o_sb[:])
                ptm_prev = ptm_cur
                pef_prev = pef_cur
                bs_prev = bs
                off_prev = off
    nc.finalize()
    return nc


def _get_nc():
    key = tuple(sorted(CFG.items()))
    if key not in _CACHE:
        _CACHE[key] = _build_nc(CFG)
    return _CACHE[key]


def make_in_maps(input, adj):
    """Host-side shard/layout prep: one input map per core."""
    import ml_dtypes

    input = np.asarray(input, dtype=np.float32)
    adj = np.asarray(adj)

    in_maps = []
    for core in range(8):
        b, h = core // 2, core % 2
        xb = input[b]                                    # [N, D]
        xs = (xb.T * SQ_ALPHA).astype(np.float16)        # pre-scaled scores
        xt = np.ascontiguousarray(xs)
        xtr = np.ascontiguousarray(xs[:, h * R:(h + 1) * R])
        xaug = np.concatenate([xb, np.ones((N, 1), np.float32)], axis=1)
        xaug = np.ascontiguousarray(xaug).astype(ml_dtypes.bfloat16)
        s = adj[b][h * R:(h + 1) * R, :] > 0             # [R rows, N cols]
        # words[rb, p, k, j]: bit i = mask row rb*512+i*64+j, key k*128+p
        sb = s[:, : NK_M * 128].reshape(NRB, 8, 64, NK_M, 128)
        words = (
            sb.astype(np.uint16)
            << np.arange(8, dtype=np.uint16)[None, :, None, None, None]
        ).sum(axis=1, dtype=np.uint16)                   # [rb, j, k, p]
        words = np.ascontiguousarray(words.transpose(0, 3, 2, 1))  # [rb,p,k,j]
        # madd[p, kt, r] for keys 3072..4095 (sg3): additive fast-exp mask
        rows = np.arange(R)
        brow = ((rows % RB) // 64).astype(np.float64)    # bit index per row
        base = FEXP_C + 1024.0 * brow                    # [R]
        m3 = s[:, NK_M * 128:]                           # [R, 1024]
        madd = np.where(m3, base[:, None], -32768.0)     # [R, 1024]
        if h == 1:
            # diagonal keys (global row == key) live in sg3 for local
            # rows >= 1024; clamp so psS_diag + madd stays below the
            # fp16 inf bit region (31744)
            xs64 = xs.astype(np.float64)
            g = np.arange(3072, 4096)                    # global keys in sg3
            r_idx = g - R                                # local row = g-2048
            ps_diag = (xs64[:, g] * xs64[:, g]).sum(axis=0)
            cap = 31500.0 - ps_diag
            col = g - NK_M * 128
            cur = madd[r_idx, col]
            madd[r_idx, col] = np.where(
                m3[r_idx, col], np.minimum(cur, cap), cur
            )
        madd = madd.reshape(R, SG, 128).transpose(2, 1, 0)  # [p, kt, r]
        madd = np.ascontiguousarray(np.round(madd)).astype(np.int16)
        in_maps.append({
            "xt": xt, "xtr": xtr, "xaug": xaug,
            "words": words, "madd": madd,
        })
    return in_maps


def run_device(in_maps, trace=False, trace_cores=None):
    import concourse.bass_utils as bass_utils

    if trace:
        bass_utils.upload_artifacts = lambda tmpdir: ""  # no bucket in sandbox
    nc = _get_nc()
    return bass_utils.run_bass_kernel_spmd(
        nc, in_maps, list(range(8)), trace=trace, trace_cores=trace_cores
    )


def kernel(input, adj):
    res = run_device(make_in_maps(input, adj))
    out = np.empty((B, N, D), dtype=np.float32)
    for core in range(8):
        b, h = core // 2, core % 2
        out[b, h * R:(h + 1) * R, :] = res.results[core]["o"]
    return out


# revision 9
# speedup vs baseline: 1.7137x; 1.3051x over previous
"""GAT-style masked self-attention (B=4, N=4096, D=128) on 8 trn2 NeuronCores.

reference:
    scores = X @ X^T / sqrt(D)            [B, N, N]
    masked = where(adj > 0, scores, -1e12)
    attn   = softmax(masked, axis=2)
    out    = attn @ X                     [B, N, D]

Sharding: 8 cores <- (batch b, row-half h); each core handles 2048 rows
of one batch element against all 4096 keys. No collectives.

Device algorithm (per core), orientation "S^T" (keys on partitions).
X is host-prescaled by sqrt(ALPHA) so the score matmul's PSUM value is
directly the fast-exp fixed-point argument psS = raw_dot*ALPHA.

Per row-block, key super-groups are processed in order (3, 0, 1, 2):

  sg 3 (keys 3072:4095), DVE fast-exp with the mask baked in:
      u16 = convert(psS + madd)   [one tensor_tensor add]
    madd (host int16) = unmasked ? (C + 1024*bit(row)) : -32768.
    Unmasked: Schraudolph fast exp -- the u16 bit pattern read as fp16
    is 2^bit(row) * exp(raw_dot*SCALE - 8); masked: the sum is negative
    and the u16 convert clamps to 0. The host clamps diagonal entries
    so the sum stays below the fp16 inf bit region. The result feeds
    the AV matmul directly (bitcast fp16), no separate mask multiply.

  sg 0..2 (keys 0:3071), ACT exp + bit-packed mask:
    - ACT evicts PSUM with exp fused: p = exp(psS*(SCALE/ALPHA) - 8),
      output bfloat16 (wide exponent absorbs the mask row constants)
    - mask host-packed 8 rows per uint16 word (bit i of word j in a
      512-row chunk = row i*64+j); DVE unpacks with one tensor_scalar
      bitwise_and per bit (4x mode): m = words & (1<<i) in {0, 2^i}
    - ptm = p * m via mixed tensor_tensor mult (bf16 x u16 -> bf16, 2x),
      split between DVE and GPSIMD

  The per-row constant 2^bit(row) is identical across all keys of a row
  (additively in sg3's exponent, multiplicatively in sg0-2), so it
  cancels in the softmax ratio.

  AV matmul with the denominator fused via an appended ones-column:
      psO[rc] (+)= w_k.T @ [X_k | 1] over k in (24..31, 0..23)
      w_k = pef (fp16 bitcast) for sg3, ptm (bf16) for sg0..2
      out = psO[:, :128] * (1 / psO[:, 128]) -- recip on DVE, the
      normalize multiply on ACT (Copy activation with scale AP)

  Row blocks are software-pipelined: block i runs scores/exp/mask while
  block i-1 runs its AV matmuls; AV matmuls are emitted first within
  each group so PE covers the eviction drain.
"""

import math
import sys

sys.path.insert(0, "/opt/trn_rl_repo")

import numpy as np

B, N, D = 4, 4096, 128
R = N // 2            # rows per core
NK = N // 128         # 32 key tiles
NK_M = 30             # key tiles masked via bit-unpack; 30,31 use fexp
RB = 512              # row granularity of the host-packed mask layout
NRB = R // RB
SG = 8                # key tiles per super group
NSG = NK // SG
NK_F = NK - NK_M      # fast-exp key tiles
SCALE = 1.0 / math.sqrt(D)
EXP_BIAS = -8.0

# fast-exp: exp(z) ~ f16_bits(round(z*1024/ln2 + 15360 - ADJ))
LOG2_SC = (1 << 10) / math.log(2.0)     # 1477.3196
ALPHA = SCALE * LOG2_SC                 # psS = raw_dot * ALPHA
SQ_ALPHA = math.sqrt(ALPHA)             # host pre-scale per score operand
FEXP_ADJ = 50.0
FEXP_C = float(round((15 << 10) + EXP_BIAS * LOG2_SC - FEXP_ADJ))
ACT_SCALE = SCALE / ALPHA               # makes ACT see raw_dot*SCALE

# row blocks (offset, size): last two halved to shorten the AV drain tail
BLOCKS = [(0, 512), (512, 512), (1024, 512), (1536, 256), (1792, 256)]

# mask-mul spans (k0, nk): GPSIMD tensor ops starve DVE of SBUF
# bandwidth (~4x slowdown on concurrent DVE ops), so all muls are DVE
MUL_SPANS = [(24, 6), (0, 8), (8, 8), (16, 8)]

CFG = dict(kg=2, psum_s_bufs=2)

_CACHE = {}


def _build_nc(cfg):
    from concourse import bacc
    import concourse.mybir as mybir
    from concourse.tile import TileContext

    dt = mybir.dt
    kg = cfg["kg"]

    nc = bacc.Bacc(None, target_bir_lowering=False)

    xt_d = nc.dram_tensor("xt", [D, N], dt.float16, kind="ExternalInput")
    xtr_d = nc.dram_tensor("xtr", [D, R], dt.float16, kind="ExternalInput")
    xaug_d = nc.dram_tensor("xaug", [N, D + 1], dt.bfloat16, kind="ExternalInput")
    # bit-packed 0/1 mask for key tiles 0..23
    w_d = nc.dram_tensor("words", [NRB, 128, NK_M, 64], dt.uint16,
                         kind="ExternalInput")
    # additive fast-exp mask for key tiles 24..31
    madd_d = nc.dram_tensor("madd", [128, NK_F, R], dt.int16,
                            kind="ExternalInput")
    o_d = nc.dram_tensor("o", [R, D], dt.float32, kind="ExternalOutput")

    with TileContext(nc) as tc:
        with (
            tc.tile_pool(name="singles", bufs=1) as singles,
            tc.tile_pool(name="words", bufs=2) as w_pool,
            tc.tile_pool(name="madd", bufs=2) as madd_pool,
            tc.tile_pool(name="mu", bufs=1) as mu_pool,
            tc.tile_pool(name="ptm", bufs=2) as ptm_pool,
            tc.tile_pool(name="pe", bufs=2) as pe_pool,
            tc.tile_pool(name="pef", bufs=2) as pef_pool,
            tc.tile_pool(name="outs", bufs=4) as out_pool,
            tc.tile_pool(name="small", bufs=4) as small_pool,
            tc.tile_pool(name="psS", bufs=cfg["psum_s_bufs"], space="PSUM") as psS_pool,
            tc.tile_pool(name="psO", bufs=4, space="PSUM") as psO_pool,
        ):
            ebias = singles.tile([128, 1], mybir.dt.float32)
            nc.vector.memset(ebias[:], EXP_BIAS)
            # warm the exp table while the init DMAs stream in
            warm = small_pool.tile([128, 1], mybir.dt.float32, tag="warm")
            nc.vector.memset(warm[:], 0.0)
            warm2 = small_pool.tile([128, 1], mybir.dt.float32, tag="warm")
            nc.scalar.activation(
                warm2[:], warm[:], mybir.ActivationFunctionType.Exp, scale=1.0
            )

            # init DMAs staggered by first consumption. sg3 (keys 3072:)
            # is processed first, so its xt slice and madd chunk lead.
            xt_sb = singles.tile([D, N], dt.float16)
            xtr_sb = singles.tile([D, R], dt.float16)
            nc.sync.dma_start(out=xtr_sb[:, 0:512], in_=xtr_d[:, 0:512])
            nc.sync.dma_start(out=xt_sb[:, 3072:4096], in_=xt_d[:, 3072:4096])
            madd_tiles = {}
            madd_tiles[0] = madd_pool.tile([128, NK_F, 512], dt.int16,
                                           tag="madd", name="madd_0")
            nc.gpsimd.dma_start(out=madd_tiles[0][:], in_=madd_d[:, :, 0:512])
            nc.sync.dma_start(out=xt_sb[:, 0:1536], in_=xt_d[:, 0:1536])
            w_tiles = {}
            w_tiles[0] = w_pool.tile([128, NK_M, 64], dt.uint16, tag="w",
                                     name="w_0")
            nc.gpsimd.dma_start(out=w_tiles[0][:], in_=w_d[0])
            nc.sync.dma_start(out=xtr_sb[:, 512:1024], in_=xtr_d[:, 512:1024])
            nc.sync.dma_start(out=xt_sb[:, 1536:3072], in_=xt_d[:, 1536:3072])
            xaug_sb = singles.tile([128, NK, D + 1], dt.bfloat16)
            nc.gpsimd.dma_start(
                out=xaug_sb[:],
                in_=xaug_d[:, :].rearrange("(t p) d -> p t d", p=128),
            )
            nc.gpsimd.dma_start(out=xtr_sb[:, 1024:2048], in_=xtr_d[:, 1024:2048])

            NB = len(BLOCKS)
            ptm_prev = None
            pef_prev = None
            bs_prev = None
            off_prev = None

            # processing order: fexp tiles (30,31) first, then 24..29,
            # then 0..23 -- the DVE fast-exp group leads each block
            K_ORDER = [30, 31, 24, 25, 26, 27, 28, 29] + list(range(24))
            K_FIRST, K_LAST = K_ORDER[0], K_ORDER[-1]

            def av_k_seq():
                return iter(K_ORDER)

            def emit_av(psO, k, rc):
                if k >= NK_M:
                    lhsT = pef_prev[:, k - NK_M, rc * 128:(rc + 1) * 128].bitcast(
                        dt.float16
                    )
                else:
                    lhsT = ptm_prev[:, k, rc * 128:(rc + 1) * 128]
                nc.tensor.matmul(
                    psO[rc][:, :],
                    lhsT=lhsT,
                    rhs=xaug_sb[:, k, :],
                    start=(k == K_FIRST),
                    stop=(k == K_LAST),
                )

            for phase in range(NB + 1):
                ptm_cur = None
                pef_cur = None
                psO = None
                if phase < NB:
                    off, bs = BLOCKS[phase]
                    rb = off // RB
                    bit0 = (off % RB) // 64
                    nbits = bs // 64
                    # prefetch next block's madd / words
                    if phase + 1 < NB:
                        off_n, bs_n = BLOCKS[phase + 1]
                        madd_tiles[phase + 1] = madd_pool.tile(
                            [128, NK_F, bs_n], dt.int16, tag="madd",
                            name=f"madd_{phase + 1}"
                        )
                        nc.gpsimd.dma_start(
                            out=madd_tiles[phase + 1][:],
                            in_=madd_d[:, :, off_n:off_n + bs_n],
                        )
                        rb_n = off_n // RB
                        if rb_n != rb and rb_n not in w_tiles:
                            w_tiles[rb_n] = w_pool.tile(
                                [128, NK_M, 64], dt.uint16, tag="w",
                                name=f"w_{rb_n}"
                            )
                            nc.gpsimd.dma_start(out=w_tiles[rb_n][:],
                                                in_=w_d[rb_n])
                    # unpack this block's mask bits (key tiles 0..23)
                    m_u = mu_pool.tile([128, NK_M, bs], dt.uint16, tag="mu",
                                       name=f"mu_{phase}")
                    wt = w_tiles[rb]
                    for i in range(nbits):
                        nc.vector.tensor_scalar(
                            m_u[:, :, i * 64:(i + 1) * 64],
                            wt[:],
                            1 << (bit0 + i),
                            None,
                            mybir.AluOpType.bitwise_and,
                        )
                    ptm_cur = ptm_pool.tile([128, NK_M, bs], dt.bfloat16,
                                            tag="ptm", name=f"ptm_{phase}")
                    pef_cur = pef_pool.tile([128, SG, bs], dt.uint16,
                                            tag="pef", name=f"pef_{phase}")
                    madd_t = madd_tiles.get(phase)
                if phase >= 1:
                    psO = [
                        psO_pool.tile(
                            [128, D + 1], mybir.dt.float32,
                            tag="psO", name=f"psO_{phase}_{rc}",
                        )
                        for rc in range(bs_prev // 128)
                    ]

                if phase == NB:
                    # drain: rc-major AV bursts so each psO finishes early
                    for rc in range(bs_prev // 128):
                        for k in av_k_seq():
                            emit_av(psO, k, rc)
                        recip = small_pool.tile([128, 1], mybir.dt.float32,
                                                tag="recip", name=f"recipd_{rc}")
                        nc.vector.reciprocal(recip[:], psO[rc][:, D:D + 1])
                        o_sb = out_pool.tile([128, D], mybir.dt.float32, tag="o",
                                             name=f"od_{rc}")
                        nc.scalar.activation(
                            o_sb[:], psO[rc][:, 0:D],
                            mybir.ActivationFunctionType.Copy, scale=recip[:],
                        )
                        r0 = off_prev + rc * 128
                        nc.sync.dma_start(out=o_d[r0:r0 + 128, :], in_=o_sb[:])
                    break

                kg_b = kg * (RB // bs)   # keep kg_b*bs = 1024 elems per evict
                halves = 2 if bs == 512 else 1
                hs = bs // halves
                av_iter = av_k_seq()
                groups = [K_ORDER[i:i + kg_b] for i in range(0, NK, kg_b)]
                pe_span = {
                    k0: pe_pool.tile([128, nkk, bs], dt.bfloat16,
                                     tag=f"pe{k0}", name=f"pe_{phase}_{k0}")
                    for k0, nkk in MUL_SPANS
                }
                span_of = {}
                for k0, nkk in MUL_SPANS:
                    for k in range(k0, k0 + nkk):
                        span_of[k] = k0
                done = set()
                muls_emitted = set()
                for gi, gks in enumerate(groups):
                    # AV matmuls for the previous block first: PE has work
                    # while the eviction drains this group's scores
                    if phase >= 1:
                        for _ in range(kg_b):
                            k = next(av_iter)
                            for rc in range(bs_prev // 128):
                                emit_av(psO, k, rc)
                    ps = psS_pool.tile([128, kg_b, bs], mybir.dt.float32,
                                       tag="psS", name=f"psS_{phase}_{gi}")
                    for j, k in enumerate(gks):
                        nc.tensor.matmul(
                            ps[:, j, :],
                            lhsT=xt_sb[:, k * 128:(k + 1) * 128],
                            rhs=xtr_sb[:, off:off + bs],
                            start=True,
                            stop=True,
                        )
                    # evict in engine/span-contiguous runs: DVE fast-exp
                    # (mask baked into madd) or ACT exp
                    j = 0
                    while j < len(gks):
                        if gks[j] >= NK_M:
                            j2 = j
                            while j2 < len(gks) and gks[j2] >= NK_M:
                                j2 += 1
                            f0 = gks[j] - NK_M
                            f1 = gks[j2 - 1] - NK_M + 1
                            nc.vector.tensor_tensor(
                                pef_cur[:, f0:f1, :],
                                ps[:, j:j2, :],
                                madd_t[:, f0:f1, :],
                                mybir.AluOpType.add,
                            )
                        else:
                            k0s = span_of[gks[j]]
                            j2 = j
                            while (j2 < len(gks) and gks[j2] < NK_M
                                   and span_of[gks[j2]] == k0s
                                   and gks[j2] - gks[j] == j2 - j):
                                j2 += 1
                            i0 = gks[j] - k0s
                            nc.scalar.activation(
                                pe_span[k0s][:, i0:i0 + (j2 - j), :],
                                ps[:, j:j2, :],
                                mybir.ActivationFunctionType.Exp,
                                bias=ebias[:],
                                scale=ACT_SCALE,
                            )
                        for jj in range(j, j2):
                            done.add(gks[jj])
                        j = j2
                    # masked probs for spans whose evictions completed
                    for k0s, nkk in MUL_SPANS:
                        if k0s in muls_emitted:
                            continue
                        if all((k0s + t) in done for t in range(nkk)):
                            muls_emitted.add(k0s)
                            for h in range(halves):
                                nc.vector.tensor_tensor(
                                    ptm_cur[:, k0s:k0s + nkk,
                                            h * hs:(h + 1) * hs],
                                    pe_span[k0s][:, :, h * hs:(h + 1) * hs],
                                    m_u[:, k0s:k0s + nkk,
                                        h * hs:(h + 1) * hs],
                                    mybir.AluOpType.mult,
                                )
                if phase >= 1:
                    for rc in range(bs_prev // 128):
                        recip = small_pool.tile([128, 1], mybir.dt.float32,
                                                tag="recip",
                                                name=f"recip_{phase}_{rc}")
                        nc.vector.reciprocal(recip[:], psO[rc][:, D:D + 1])
                        o_sb = out_pool.tile([128, D], mybir.dt.float32, tag="o",
                                             name=f"o_{phase}_{rc}")
                        nc.scalar.activation(
                            o_sb[:], psO[rc][:, 0:D],
                            mybir.ActivationFunctionType.Copy, scale=recip[:],
                        )
                        r0 = off_prev + rc * 128
                        nc.sync.dma_start(out=o_d[r0:r0 + 128, :], in_=o_sb[:])
                ptm_prev = ptm_cur
                pef_prev = pef_cur
                bs_prev = bs
                off_prev = off
    nc.finalize()
    return nc


def _get_nc():
    key = tuple(sorted(CFG.items()))
    if key not in _CACHE:
        _CACHE[key] = _build_nc(CFG)
    return _CACHE[key]


def make_in_maps(input, adj):
    """Host-side shard/layout prep: one input map per core."""
    import ml_dtypes

    input = np.asarray(input, dtype=np.float32)
    adj = np.asarray(adj)

    in_maps = []
    for core in range(8):
        b, h = core // 2, core % 2
        xb = input[b]                                    # [N, D]
        xs = (xb.T * SQ_ALPHA).astype(np.float16)        # pre-scaled scores
        xt = np.ascontiguousarray(xs)
        xtr = np.ascontiguousarray(xs[:, h * R:(h + 1) * R])
        xaug = np.concatenate([xb, np.ones((N, 1), np.float32)], axis=1)
        xaug = np.ascontiguousarray(xaug).astype(ml_dtypes.bfloat16)
        s = adj[b][h * R:(h + 1) * R, :] > 0             # [R rows, N cols]
        # words[rb, p, k, j]: bit i = mask row rb*512+i*64+j, key k*128+p
        sb = s[:, : NK_M * 128].reshape(NRB, 8, 64, NK_M, 128)
        words = (
            sb.astype(np.uint16)
            << np.arange(8, dtype=np.uint16)[None, :, None, None, None]
        ).sum(axis=1, dtype=np.uint16)                   # [rb, j, k, p]
        words = np.ascontiguousarray(words.transpose(0, 3, 2, 1))  # [rb,p,k,j]
        # madd[p, kt, r] for keys 3072..4095 (sg3): additive fast-exp mask
        rows = np.arange(R)
        brow = ((rows % RB) // 64).astype(np.float64)    # bit index per row
        base = FEXP_C + 1024.0 * brow                    # [R]
        m3 = s[:, NK_M * 128:]                           # [R, 1024]
        madd = np.where(m3, base[:, None], -32768.0)     # [R, 1024]
        if h == 1:
            # diagonal keys (global row == key) in the fexp range; clamp
            # so psS_diag + madd stays below the fp16 inf bit region
            xs64 = xs.astype(np.float64)
            g = np.arange(NK_M * 128, 4096)              # global fexp keys
            r_idx = g - R                                # local row = g-2048
            ps_diag = (xs64[:, g] * xs64[:, g]).sum(axis=0)
            cap = 31500.0 - ps_diag
            col = g - NK_M * 128
            cur = madd[r_idx, col]
            madd[r_idx, col] = np.where(
                m3[r_idx, col], np.minimum(cur, cap), cur
            )
        madd = madd.reshape(R, NK_F, 128).transpose(2, 1, 0)  # [p, kt, r]
        madd = np.ascontiguousarray(np.round(madd)).astype(np.int16)
        in_maps.append({
            "xt": xt, "xtr": xtr, "xaug": xaug,
            "words": words, "madd": madd,
        })
    return in_maps


def run_device(in_maps, trace=False, trace_cores=None):
    import concourse.bass_utils as bass_utils

    if trace:
        bass_utils.upload_artifacts = lambda tmpdir: ""  # no bucket in sandbox
    nc = _get_nc()
    return bass_utils.run_bass_kernel_spmd(
        nc, in_maps, list(range(8)), trace=trace, trace_cores=trace_cores
    )


def kernel(input, adj):
    res = run_device(make_in_maps(input, adj))
    out = np.empty((B, N, D), dtype=np.float32)
    for core in range(8):
        b, h = core // 2, core % 2
        out[b, h * R:(h + 1) * R, :] = res.results[core]["o"]
    return out
